# revision 1
# baseline (speedup 1.0000x reference)
"""Trainium2 Bass kernel for a LocalTransformerBlock (windowed causal attention
+ GEGLU FFN), SPMD over 8 NeuronCores.

Sharding: sequence-parallel. B=2 batches x 8 windows of 512 = 16 windows; each
core owns 2 consecutive windows (1024 tokens) of one batch and recomputes k/v
for a 512-token halo (the preceding window), so no collectives are needed.
First-window cores receive a zero halo plus an exp-bias of -1e30 that hard
masks the (nonexistent) previous window.

Layout: "feature-major" - activations live in SBUF as [feature, token] so every
matmul chains without transposes (out = lhsT.T @ rhs with weights natural and
activations as the moving operand). The host transposes x on the way in and the
per-core [DIM, 1024] output back on the way out.

Attention per (window, head): S^T[key, q] = khat.T-slice @ qhat-slice; causal
mask enters as an additive [128,128] triangular tile on the diagonal block only
(other blocks are either fully allowed or skipped via column restriction);
softmax skips the max-subtraction (scores are O(1) by construction); the value
matrix carries an appended ones-column so the attention row-sums fall out of
the same PSUM accumulation; division by the sums happens on the [64, q] output.

Numerics: bf16 matmul operands with fp32 PSUM accumulation; fp32 residual
stream; fp32r (TF32) only for layernorm statistic/broadcast matmuls.
"""

import sys

import numpy as np

sys.path.insert(0, "/opt/trn_rl_repo")

import ml_dtypes

BF = ml_dtypes.bfloat16

B, N, DIM = 2, 4096, 1024
HEADS, DH, W = 16, 64, 512
INNER = 2730
INNER_PAD = 2816  # 22 * 128
NIC = INNER_PAD // 128  # 22 inner chunks
NCORES = 8
TOK = 1024  # own tokens per core
TOKH = 1536  # incl 512-token halo
EPS = 1e-5
NEG = -1.0e30
P = 128

_CACHE = {}


def _build_program(need_ln1_bias: bool):
    def T(pool, shape, dtype, tag, **kw):
        return pool.tile(shape, dtype, name=tag, tag=tag, **kw)

    import concourse.bass as bass
    import concourse.tile as tile
    from concourse import bacc, mybir

    f32 = mybir.dt.float32
    bf16 = mybir.dt.bfloat16
    AF = mybir.ActivationFunctionType
    ALU = mybir.AluOpType
    ts = bass.ts

    nc = bacc.Bacc("TRN2", target_bir_lowering=False, debug=False,
                   num_devices=NCORES)

    # ---------------- DRAM I/O ----------------
    x_td = nc.dram_tensor("x_t", [DIM, TOKH], f32, kind="ExternalInput").ap()
    wqk_d = nc.dram_tensor("wqk", [DIM, 2048], bf16, kind="ExternalInput").ap()
    wv_d = nc.dram_tensor("wv", [DIM, 1024], bf16, kind="ExternalInput").ap()
    wo_d = nc.dram_tensor("wo", [HEADS * DH, DIM], bf16, kind="ExternalInput").ap()
    wf1_d = nc.dram_tensor("wf1", [DIM, 2 * INNER_PAD], bf16, kind="ExternalInput").ap()
    wf2_d = nc.dram_tensor("wf2", [INNER_PAD, DIM], bf16, kind="ExternalInput").ap()
    cosq_d = nc.dram_tensor("cos_q", [P, TOK], bf16, kind="ExternalInput").ap()
    sinq_d = nc.dram_tensor("sin_q", [P, TOK], bf16, kind="ExternalInput").ap()
    cosk_d = nc.dram_tensor("cos_k", [P, TOKH], bf16, kind="ExternalInput").ap()
    sink_d = nc.dram_tensor("sin_k", [P, TOKH], bf16, kind="ExternalInput").ap()
    tri_d = nc.dram_tensor("tri", [P, P], f32, kind="ExternalInput").ap()
    pb_d = nc.dram_tensor("pbias", [P, 1], f32, kind="ExternalInput").ap()
    rotm_d = nc.dram_tensor("rotm", [P, P], bf16, kind="ExternalInput").ap()
    if need_ln1_bias:
        bqk_d = nc.dram_tensor("b_qk", [P, 16], f32, kind="ExternalInput").ap()
        bv_d = nc.dram_tensor("b_v", [64, 16], f32, kind="ExternalInput").ap()
    out_td = nc.dram_tensor("out_t", [DIM, TOK], f32, kind="ExternalOutput").ap()

    with tile.TileContext(nc) as tc:
        # ---- pools; left/right sides give two independent LIFO stacks ----
        perm = tc.alloc_tile_pool(name="perm", bufs=1)
        wpool = tc.alloc_tile_pool(name="wpool", bufs=4)
        misc = tc.alloc_tile_pool(name="misc", bufs=5)
        xts_pool = tc.alloc_tile_pool(name="xts", bufs=8)
        sq_pool = tc.alloc_tile_pool(name="sqp", bufs=3)
        rtmp = tc.alloc_tile_pool(name="rtmp", bufs=4)
        ptp = tc.alloc_tile_pool(name="ptp", bufs=6)
        gtp = tc.alloc_tile_pool(name="gtp", bufs=3)
        otp = tc.alloc_tile_pool(name="otp", bufs=4)
        ps = tc.alloc_tile_pool(name="ps", bufs=5, space="PSUM")
        aux = tc.alloc_tile_pool(name="aux", bufs=3, space="PSUM")

        # ---------- permanent small tiles ----------
        ones128f = T(perm, [P, 1], f32, "ones128f")
        nc.vector.memset(ones128f, 1.0)
        ones128b = T(perm, [P, 1], bf16, "ones128b")
        nc.vector.memset(ones128b, 1.0)
        ones1f = T(perm, [1, P], f32, "ones1f")
        nc.vector.memset(ones1f, 1.0)
        ones1b = T(perm, [1, P], bf16, "ones1b")
        nc.vector.memset(ones1b, 1.0)
        onesA = T(perm, [P, 64], bf16, "onesA")
        nc.vector.memset(onesA, 1.0)
        eps_ap = T(perm, [1, 1], f32, "eps")
        nc.vector.memset(eps_ap, EPS)
        tri = T(perm, [P, P], f32, "tri")
        nc.sync.dma_start(out=tri, in_=tri_d)
        pb = T(perm, [P, 1], f32, "pb")
        nc.sync.dma_start(out=pb, in_=pb_d)
        rotm = T(perm, [P, P], bf16, "rotm")
        nc.sync.dma_start(out=rotm, in_=rotm_d)
        cosq = T(perm, [P, TOK], bf16, "cosq")
        nc.sync.dma_start(out=cosq, in_=cosq_d)
        sinq = T(perm, [P, TOK], bf16, "sinq")
        nc.sync.dma_start(out=sinq, in_=sinq_d)
        cosk = T(perm, [P, TOKH], bf16, "cosk")
        nc.sync.dma_start(out=cosk, in_=cosk_d)
        sink = T(perm, [P, TOKH], bf16, "sink")
        nc.sync.dma_start(out=sink, in_=sink_d)
        if need_ln1_bias:
            bqk = T(perm, [P, 16], f32, "bqk")
            nc.sync.dma_start(out=bqk, in_=bqk_d)
            bv = T(perm, [64, 16], f32, "bv")
            nc.sync.dma_start(out=bv, in_=bv_d)

        def layernorm_fm(src, dst_tiles, ntch):
            """Feature-major LN. src: DRAM AP [DIM, ntok] or list of 8 fp32
            SBUF tiles. dst_tiles: 8 bf16 tiles. ntch: 512-token chunks."""
            from_dram = not isinstance(src, list)

            for t in range(ntch):
                tc512 = ts(t, 512)
                chunks = []
                for c in range(8):
                    if from_dram:
                        xt = T(xts_pool, [P, 512], f32, "xts")
                        nc.sync.dma_start(out=xt, in_=src[ts(c, P), tc512])
                    else:
                        xt = src[c][:, tc512]
                    chunks.append(xt)

                def chunk(c, _tc):
                    return chunks[c]

                s_ps = T(ps, [1, 512], f32, "ps")
                ss_ps = T(ps, [1, 512], f32, "ps")
                for c in range(8):
                    xt = chunk(c, tc512)
                    sq = T(sq_pool, [P, 512], bf16, "sq")
                    nc.scalar.activation(out=sq, in_=xt, func=AF.Square)
                    xb = T(sq_pool, [P, 512], bf16, "xb")
                    nc.gpsimd.tensor_copy(out=xb, in_=xt)
                    nc.tensor.matmul(s_ps, ones128b, xb,
                                     start=(c == 0), stop=(c == 7))
                    nc.tensor.matmul(ss_ps, ones128b, sq,
                                     start=(c == 0), stop=(c == 7))
                mean = T(misc, [1, 512], bf16, "stat")
                nc.scalar.activation(out=mean, in_=s_ps, func=AF.Copy,
                                     scale=1.0 / DIM)
                msq = T(misc, [1, 512], f32, "stat")
                nc.scalar.activation(out=msq, in_=mean, func=AF.Square)
                var = T(misc, [1, 512], f32, "stat")
                nc.vector.scalar_tensor_tensor(
                    out=var, in0=ss_ps, scalar=1.0 / DIM, in1=msq,
                    op0=ALU.mult, op1=ALU.subtract)
                std = T(misc, [1, 512], f32, "stat")
                nc.scalar.activation(out=std, in_=var, func=AF.Sqrt,
                                     bias=eps_ap)
                rstd = T(misc, [1, 512], f32, "stat")
                nc.vector.reciprocal_approx_fast(out=rstd, in_=std)
                mb = T(aux, [P, 512], f32, "aux")
                nc.tensor.matmul(mb, ones1b, mean,
                                 start=True, stop=True)
                rb = T(aux, [P, 512], f32, "aux")
                nc.tensor.matmul(rb, ones1f, rstd,
                                 start=True, stop=True)
                for c in range(8):
                    xt = chunk(c, tc512)
                    tmp = T(sq_pool, [P, 512], f32, "lntmp")
                    nc.vector.tensor_sub(tmp, xt, mb)
                    nc.vector.tensor_mul(dst_tiles[c][:, tc512], tmp, rb)

        # ================= P1: LN1 (x streamed from DRAM) ==============
        h1_pool = tc.alloc_tile_pool(name="h1", bufs=1, side="left")
        h1 = [T(h1_pool, [P, TOKH], bf16, f"h1_{c}") for c in range(8)]
        layernorm_fm(x_td, h1, 3)

        # ================= P2: Q,K projections + RoPE ==================
        att_pool = tc.alloc_tile_pool(name="att", bufs=1, side="right")
        qhat = [T(att_pool, [P, TOK], bf16, f"qh_{f}") for f in range(8)]
        khat = [T(att_pool, [P, TOKH], bf16, f"kh_{f}") for f in range(8)]
        vv = [T(att_pool, [P, 16 * 65], bf16, f"vv_{k}")
              for k in range(12)]

        def rope(src_ps, fcol, dst, dcols, cos, sin, ccols):
            # dst[:, dcols] = src*cos + (rot64 @ src)*sin  (+ bias pre-rot)
            qsb = T(sq_pool, [P, 512], bf16, "qsb")
            if need_ln1_bias:
                nc.scalar.activation(out=qsb, in_=src_ps, func=AF.Identity,
                                     bias=bqk[:, fcol:fcol + 1])
            else:
                nc.scalar.copy(qsb, src_ps)
            rot_ps = T(ps, [P, 512], f32, "ps")
            nc.tensor.matmul(rot_ps, rotm, qsb, start=True, stop=True)
            s1 = T(rtmp, [P, 512], bf16, "s1")
            nc.vector.tensor_mul(s1, qsb, cos[:, ccols])
            s2 = T(rtmp, [P, 512], bf16, "s2")
            nc.vector.tensor_mul(s2, rot_ps, sin[:, ccols])
            nc.vector.tensor_add(dst[:, dcols], s1, s2)

        for fp in range(8):  # 256-wide fout stripes; 0-3 = q, 4-7 = k
            wt = T(wpool, [P, 8, 256], bf16, "w")
            nc.sync.dma_start(
                out=wt,
                in_=wqk_d[:, ts(fp, 256)].rearrange("(c p) m -> p c m", p=P))
            for sub in range(2):
                f = 2 * fp + sub
                is_q = fp < 4
                for t in range(2 if is_q else 3):
                    qps = T(ps, [P, 512], f32, "ps")
                    hcol = slice(512 + t * 512, 1024 + t * 512) if is_q \
                        else slice(t * 512, (t + 1) * 512)
                    for c in range(8):
                        nc.tensor.matmul(
                            qps, wt[:, c, ts(sub, P)], h1[c][:, hcol],
                            start=(c == 0), stop=(c == 7))
                    if is_q:
                        rope(qps, f, qhat[f], ts(t, 512), cosq, sinq,
                             ts(t, 512))
                    else:
                        rope(qps, f, khat[f - 8], ts(t, 512), cosk, sink,
                             ts(t, 512))

        # ================= P2b: V (token-major) + ones column ==========
        for k in range(12):
            nc.gpsimd.memset(vv[k], 1.0)
        vvv = [vv[k].rearrange("p (h x) -> p h x", h=16) for k in range(12)]
        wvt = []
        for vh in range(2):  # 512-wide v column stripes = 8 heads each
            wt = T(wpool, [P, 8, 512], bf16, "w")
            nc.sync.dma_start(
                out=wt,
                in_=wv_d[:, ts(vh, 512)].rearrange("(c p) m -> p c m", p=P))
            wvt.append(wt)
        for k in range(12):  # key-chunk outer: vv[0..7] complete first so
            for vh in range(2):  # window-0 attention can begin early
                vps = T(ps, [P, 512], f32, "ps")
                for c in range(8):
                    nc.tensor.matmul(
                        vps, h1[c][:, ts(k, P)], wvt[vh][:, c, :],
                        start=(c == 0), stop=(c == 7))
                dst = vvv[k][:, 8 * vh:8 * vh + 8, 0:64]
                nc.scalar.copy(dst, vps.rearrange("p (h x) -> p h x", h=8))
        h1_pool.release()

        # ================= P3: windowed attention ======================
        oh_pool = tc.alloc_tile_pool(name="oh", bufs=1, side="left")
        o_head = [T(oh_pool, [64, TOK], bf16, f"oh_{h}")
                  for h in range(HEADS)]
        wo_t = []
        for d in range(8):  # prefetch out-proj stripes during attention
            wt = T(wpool, [64, 16, P], bf16, "w")
            nc.sync.dma_start(
                out=wt,
                in_=wo_d[:, ts(d, P)].rearrange("(h q) m -> q h m", q=64))
            wo_t.append(wt)

        for lw in range(2):
            for h in range(HEADS):
                hp, hb = h // 2, 64 * (h % 2)
                o_ps = T(aux, [65, 512], f32, "aux")
                for kc8 in range(8):
                    g = lw * 4 + kc8
                    own = kc8 >= 4
                    q0 = (kc8 - 4) * P if own else 0
                    nq = 512 - q0
                    S = T(ps, [P, 512], f32, "ps")
                    nc.tensor.matmul(
                        S[:, :nq],
                        khat[hp][hb:hb + 64, ts(g, P)],
                        qhat[hp][hb:hb + 64, lw * 512 + q0:(lw + 1) * 512],
                        start=True, stop=True)
                    if own:
                        nc.vector.tensor_add(S[:, 0:P], S[:, 0:P], tri)
                    Pt = T(ptp, [P, 512], bf16, "pt")
                    bias = pb if (lw == 0 and not own) else 0.0
                    nc.scalar.activation(out=Pt[:, :nq], in_=S[:, :nq],
                                         func=AF.Exp, bias=bias)
                    nc.tensor.matmul(
                        o_ps[:, q0:512],
                        vv[g][:, h * 65: h * 65 + 65],
                        Pt[:, :nq],
                        start=(kc8 == 0), stop=(kc8 == 7),
                        skip_group_check=True)
                # normalize: o[0:64] * (1 / sums) with sums in row 64
                ssb = T(misc, [65, 512], bf16, "stat")
                nc.vector.tensor_copy(ssb[64:65, :], o_ps[64:65, :])
                bc = T(aux, [64, 512], f32, "aux")
                nc.tensor.matmul(bc, onesA[64:65, :],
                                 ssb[64:65, :], start=True, stop=True)
                bcr = T(misc, [64, 512], f32, "stat")
                nc.vector.reciprocal_approx_fast(out=bcr, in_=bc)
                nc.vector.tensor_mul(o_head[h][:, ts(lw, 512)],
                                     o_ps[0:64, :], bcr)
                if need_ln1_bias:
                    nc.vector.tensor_scalar_add(
                        out=o_head[h][:, ts(lw, 512)],
                        in0=o_head[h][:, ts(lw, 512)],
                        scalar1=bv[:, h:h + 1])
        att_pool.release()

        # ================= P4: output proj + residual ==================
        x2_pool = tc.alloc_tile_pool(name="x2", bufs=1, side="right")
        x2 = [T(x2_pool, [P, TOK], f32, f"x2_{c}") for c in range(8)]
        for d in range(8):  # 128-wide output stripes
            wt = wo_t[d]
            for t in range(2):
                yps = T(ps, [P, 512], f32, "ps")
                for h in range(HEADS):
                    nc.tensor.matmul(
                        yps, wt[:, h, :], o_head[h][:, ts(t, 512)],
                        start=(h == 0), stop=(h == HEADS - 1))
                xo = T(xts_pool, [P, 512], f32, "xts")
                nc.sync.dma_start(
                    out=xo,
                    in_=x_td[ts(d, P), 512 + t * 512: 1024 + t * 512])
                nc.vector.tensor_add(x2[d][:, ts(t, 512)], yps, xo)
        oh_pool.release()

        # ================= P5: LN2 =====================================
        h2_pool = tc.alloc_tile_pool(name="h2", bufs=1, side="left")
        h2 = [T(h2_pool, [P, TOK], bf16, f"h2_{c}") for c in range(8)]
        layernorm_fm(x2, h2, 2)

        # ================= P6: FFN (GEGLU) =============================
        e_pool = tc.alloc_tile_pool(name="e", bufs=1, side="right")
        e = [T(e_pool, [P, TOK], bf16, f"e_{i}") for i in range(NIC)]
        for ip in range(NIC // 2):  # 256-wide stripes of a and g halves
            wa = T(wpool, [P, 8, 256], bf16, "w")
            nc.sync.dma_start(
                out=wa,
                in_=wf1_d[:, ts(ip, 256)].rearrange("(c p) m -> p c m", p=P))
            wg = T(wpool, [P, 8, 256], bf16, "w")
            nc.sync.dma_start(
                out=wg,
                in_=wf1_d[:, INNER_PAD + ip * 256: INNER_PAD + (ip + 1) * 256]
                .rearrange("(c p) m -> p c m", p=P))
            for sub in range(2):
                i = 2 * ip + sub
                for t in range(2):
                    aps = T(ps, [P, 512], f32, "ps")
                    gps = T(ps, [P, 512], f32, "ps")
                    for c in range(8):
                        nc.tensor.matmul(aps, wa[:, c, ts(sub, P)],
                                         h2[c][:, ts(t, 512)],
                                         start=(c == 0), stop=(c == 7))
                    for c in range(8):
                        nc.tensor.matmul(gps, wg[:, c, ts(sub, P)],
                                         h2[c][:, ts(t, 512)],
                                         start=(c == 0), stop=(c == 7))
                    gt = T(gtp, [P, 512], bf16, "gt")
                    nc.scalar.activation(out=gt, in_=gps, func=AF.Gelu)
                    nc.vector.tensor_mul(e[i][:, ts(t, 512)], aps, gt)
        h2_pool.release()

        # ff2 + residual + store
        for d in range(8):
            wt = T(wpool, [P, NIC, P], bf16, "w")
            nc.sync.dma_start(
                out=wt,
                in_=wf2_d[:, ts(d, P)].rearrange("(c p) m -> p c m", p=P))
            for t in range(2):
                yps = T(ps, [P, 512], f32, "ps")
                for i in range(NIC):
                    nc.tensor.matmul(yps, wt[:, i, :], e[i][:, ts(t, 512)],
                                     start=(i == 0), stop=(i == NIC - 1))
                ot = T(otp, [P, 512], f32, "ot")
                nc.vector.tensor_add(ot, yps, x2[d][:, ts(t, 512)])
                nc.sync.dma_start(out=out_td[ts(d, P), ts(t, 512)], in_=ot)
        e_pool.release()
        x2_pool.release()

        # release the perm-ish pools in reverse creation order (LIFO)
        aux.release()
        ps.release()
        otp.release()
        gtp.release()
        ptp.release()
        rtmp.release()
        sq_pool.release()
        xts_pool.release()
        misc.release()
        wpool.release()
        perm.release()

    nc.compile()
    return nc


def _host_prep(inputs):
    x = np.asarray(inputs["x"], np.float32)
    ln1_w = np.asarray(inputs["ln1_w"], np.float32)
    ln1_b = np.asarray(inputs["ln1_b"], np.float32)
    w_qkv = np.asarray(inputs["w_qkv"], np.float32)
    w_out = np.asarray(inputs["w_out"], np.float32)
    ln2_w = np.asarray(inputs["ln2_w"], np.float32)
    w_ff1 = np.asarray(inputs["w_ff1"], np.float32)
    w_ff2 = np.asarray(inputs["w_ff2"], np.float32)

    need_ln1_bias = bool(np.any(ln1_b != 0.0))

    wq_eff = w_qkv * ln1_w[:, None]
    wqk = np.ascontiguousarray(wq_eff[:, :2048]).astype(BF)
    wv = np.ascontiguousarray(wq_eff[:, 2048:]).astype(BF)
    wo = w_out.astype(BF)
    wf1 = np.zeros((DIM, 2 * INNER_PAD), np.float32)
    wf1[:, :INNER] = w_ff1[:, :INNER] * ln2_w[:, None]
    wf1[:, INNER_PAD:INNER_PAD + INNER] = w_ff1[:, INNER:] * ln2_w[:, None]
    wf1 = wf1.astype(BF)
    wf2 = np.zeros((INNER_PAD, DIM), np.float32)
    wf2[:INNER] = w_ff2
    wf2 = wf2.astype(BF)

    # rotate-half as a 128x128 block-diag matrix (2 heads per 128 partitions):
    # rot(q)[m] = -q[m+32] for m<32, q[m-32] for 32<=m<64 (per 64-row head)
    rot64 = np.zeros((DH, DH), np.float32)
    for m in range(32):
        rot64[m + 32, m] = -1.0
        rot64[m, m + 32] = 1.0
    rotm = np.zeros((P, P), np.float32)
    rotm[:DH, :DH] = rot64
    rotm[DH:, DH:] = rot64
    rotm = rotm.astype(BF)

    tri = np.where(np.arange(P)[None, :] >= np.arange(P)[:, None],
                   0.0, NEG).astype(np.float32)  # [key-in-diag, q]

    b_qkv = (ln1_b @ w_qkv).astype(np.float32)  # [3072]
    b_qk = np.ascontiguousarray(b_qkv[:2048].reshape(16, P).T)  # [128,16]
    b_v = np.ascontiguousarray(b_qkv[2048:].reshape(16, 64).T)  # [64,16]

    inv = 1.0 / (10000.0 ** (np.arange(0, DH, 2, dtype=np.float64) / DH))

    in_maps = []
    for c in range(NCORES):
        b = c // 4
        pos0 = (c % 4) * TOK
        xs = np.zeros((TOKH, DIM), np.float32)
        lo = pos0 - W
        if lo < 0:
            xs[W:] = x[b, pos0:pos0 + TOK]
        else:
            xs[:] = x[b, lo:pos0 + TOK]
        x_t = np.ascontiguousarray(xs.T)

        pos_own = pos0 + np.arange(TOK, dtype=np.float64)
        pos_kv = pos0 - W + np.arange(TOKH, dtype=np.float64)

        def cs(pos, scale):
            f = pos[None, :] * inv[:, None]  # [32, T]
            f = np.concatenate([f, f], axis=0)  # [64, T]
            co = np.tile(np.cos(f) * scale, (2, 1)).astype(BF)
            si = np.tile(np.sin(f) * scale, (2, 1)).astype(BF)
            return np.ascontiguousarray(co), np.ascontiguousarray(si)

        cos_q, sin_q = cs(pos_own, DH ** -0.5)
        cos_k, sin_k = cs(pos_kv, 1.0)

        pbias = np.full((P, 1), NEG if c % 4 == 0 else 0.0, np.float32)

        m = {
            "x_t": x_t, "wqk": wqk, "wv": wv, "wo": wo,
            "wf1": wf1, "wf2": wf2,
            "cos_q": cos_q, "sin_q": sin_q, "cos_k": cos_k, "sin_k": sin_k,
            "tri": tri, "pbias": pbias, "rotm": rotm,
        }
        if need_ln1_bias:
            m["b_qk"] = b_qk
            m["b_v"] = b_v
        in_maps.append(m)
    return in_maps, need_ln1_bias


def get_program_and_inputs(inputs):
    in_maps, need_ln1_bias = _host_prep(inputs)
    key = ("prog", need_ln1_bias)
    if key not in _CACHE:
        _CACHE[key] = _build_program(need_ln1_bias)
    return _CACHE[key], in_maps


def assemble(results):
    out = np.empty((B, N, DIM), np.float32)
    for c in range(NCORES):
        b = c // 4
        pos0 = (c % 4) * TOK
        out[b, pos0:pos0 + TOK] = results[c]["out_t"].T
    return out


def run(inputs, trace=False, **kw):
    from concourse.bass_utils import run_bass_kernel_spmd
    nc, in_maps = get_program_and_inputs(inputs)
    res = run_bass_kernel_spmd(nc, in_maps, list(range(NCORES)),
                               trace=trace, **kw)
    return assemble(res.results), res


def kernel(**inputs) -> np.ndarray:
    out, _ = run(inputs, trace=False)
    return out



# revision 9
# speedup vs baseline: 1.2784x; 1.2784x over previous
"""Trainium2 Bass kernel for a LocalTransformerBlock (windowed causal attention
+ GEGLU FFN), SPMD over 8 NeuronCores.

Sharding: sequence-parallel. B=2 batches x 8 windows of 512 = 16 windows; each
core owns 2 consecutive windows (1024 tokens) of one batch and recomputes k/v
for a 512-token halo (the preceding window), so no collectives are needed.
First-window cores receive a zero halo plus an exp-bias of -1e30 that hard
masks the (nonexistent) previous window.

v2: all large GEMMs run in fp8(e4m3) with DoubleRow perf mode (2 weights per
PE cell, K=256 per accumulation step), with power-of-2 scale management
(weights x64, rescales folded into existing elementwise ops). Attention
scores S stay bf16 (head pairs occupy separate PE row groups so their S
matmuls run concurrently); P (exp scores) and V are fp8 and P@V runs
DoubleRow over key-chunk pairs. The causal mask enters via a PE-side
identity@tri accumulation step; masked P regions are persistent zeros in
dedicated Pt tiles. Softmax row sums ride along as a 16-valued column in V.

Layout: feature-major activations [feature, token]; host transposes x in and
the per-core [DIM, 1024] output back out.
"""

import sys

import numpy as np

sys.path.insert(0, "/opt/trn_rl_repo")

import ml_dtypes

BF = ml_dtypes.bfloat16

B, N, DIM = 2, 4096, 1024
HEADS, DH, W = 16, 64, 512
INNER = 2730
INNER_PAD = 2816  # 22 * 128
NIC = INNER_PAD // 128  # 22 inner chunks
NCORES = 8
TOK = 1024  # own tokens per core
TOKH = 1536  # incl 512-token halo
EPS = 1e-5
NEG = -1.0e30
P = 128
WS = 64.0  # fp8 weight scale
OS = 16.0  # fp8 o / e / v scale

_CACHE = {}


def _build_program(need_ln1_bias: bool):
    def T(pool, shape, dtype, tag, **kw):
        return pool.tile(shape, dtype, name=tag, tag=tag, **kw)

    import concourse.bass as bass
    import concourse.tile as tile
    from concourse import bacc, mybir

    f32 = mybir.dt.float32
    bf16 = mybir.dt.bfloat16
    fp8 = mybir.dt.float8e4
    AF = mybir.ActivationFunctionType
    ALU = mybir.AluOpType
    DR = mybir.MatmulPerfMode.DoubleRow
    ts = bass.ts

    nc = bacc.Bacc("TRN2", target_bir_lowering=False, debug=False,
                   num_devices=NCORES)

    # ---------------- DRAM I/O (all weights host-prepacked) ----------------
    x_td = nc.dram_tensor("x_t", [DIM, TOKH], f32, kind="ExternalInput").ap()
    wqk_d = nc.dram_tensor("wqk8", [P, 8, 2048], fp8, kind="ExternalInput").ap()
    wv_d = nc.dram_tensor("wv8", [P, 4, 2048], fp8, kind="ExternalInput").ap()
    wo_d = nc.dram_tensor("wo8", [P, 8, 1024], fp8, kind="ExternalInput").ap()
    wf1a_d = nc.dram_tensor("wf1a8", [P, NIC, 1024], fp8,
                            kind="ExternalInput").ap()
    wf1g_d = nc.dram_tensor("wf1g8", [P, NIC, 1024], fp8,
                            kind="ExternalInput").ap()
    wf2_d = nc.dram_tensor("wf28", [P, 8, 2816], fp8, kind="ExternalInput").ap()
    cosq_d = nc.dram_tensor("cos_q", [P, TOK], bf16, kind="ExternalInput").ap()
    sinq_d = nc.dram_tensor("sin_q", [P, TOK], bf16, kind="ExternalInput").ap()
    cosk_d = nc.dram_tensor("cos_k", [P, TOKH], bf16, kind="ExternalInput").ap()
    sink_d = nc.dram_tensor("sin_k", [P, TOKH], bf16, kind="ExternalInput").ap()
    tri_d = nc.dram_tensor("tri", [P, P], bf16, kind="ExternalInput").ap()
    id_d = nc.dram_tensor("idm", [P, P], bf16, kind="ExternalInput").ap()
    pb_d = nc.dram_tensor("pbias", [P, 1], f32, kind="ExternalInput").ap()
    rotm_d = nc.dram_tensor("rotm", [P, P], bf16, kind="ExternalInput").ap()
    if need_ln1_bias:
        bqk_d = nc.dram_tensor("b_qk", [1, 2048], bf16, kind="ExternalInput").ap()
        bv_d = nc.dram_tensor("b_v", [1, 1024], bf16, kind="ExternalInput").ap()
    out_td = nc.dram_tensor("out_t", [DIM, TOK], f32, kind="ExternalOutput").ap()

    with tile.TileContext(nc) as tc:
        perm = tc.alloc_tile_pool(name="perm", bufs=1)
        wpool = tc.alloc_tile_pool(name="wpool", bufs=4)
        misc = tc.alloc_tile_pool(name="misc", bufs=4)
        xts_pool = tc.alloc_tile_pool(name="xts", bufs=12)
        sq_pool = tc.alloc_tile_pool(name="sqp", bufs=4)
        rtmp = tc.alloc_tile_pool(name="rtmp", bufs=3)
        gtp = tc.alloc_tile_pool(name="gtp", bufs=3)
        otp = tc.alloc_tile_pool(name="otp", bufs=4)

        # ---------- permanent small tiles ----------
        ones128b = T(perm, [P, 1], bf16, "ones128b")
        nc.vector.memset(ones128b, 1.0)
        ones1b = T(perm, [1, P], bf16, "ones1b")
        nc.vector.memset(ones1b, 1.0)
        ones1f = T(perm, [1, P], f32, "ones1f")
        nc.vector.memset(ones1f, 1.0)
        onesr = T(perm, [1, 512], bf16, "onesr")
        nc.vector.memset(onesr, 1.0)
        selpair = T(perm, [33, P], f32, "selpair")
        nc.gpsimd.memset(selpair, 0.0)
        nc.vector.memset(selpair[0:1, 0:64], OS)
        nc.vector.memset(selpair[32:33, 64:128], OS)
        eps_ap = T(perm, [1, 1], f32, "eps")
        nc.vector.memset(eps_ap, EPS)
        tri = T(perm, [P, P], bf16, "tri")
        nc.sync.dma_start(out=tri, in_=tri_d)
        id128 = T(perm, [P, P], bf16, "id128")
        nc.sync.dma_start(out=id128, in_=id_d)
        pb = T(perm, [P, 1], f32, "pb")
        nc.sync.dma_start(out=pb, in_=pb_d)
        rotm = T(perm, [P, P], bf16, "rotm")
        nc.sync.dma_start(out=rotm, in_=rotm_d)
        cosq = T(perm, [P, TOK], bf16, "cosq")
        nc.sync.dma_start(out=cosq, in_=cosq_d)
        sinq = T(perm, [P, TOK], bf16, "sinq")
        nc.sync.dma_start(out=sinq, in_=sinq_d)
        cosk = T(perm, [P, TOKH], bf16, "cosk")
        nc.sync.dma_start(out=cosk, in_=cosk_d)
        sink = T(perm, [P, TOKH], bf16, "sink")
        nc.sync.dma_start(out=sink, in_=sink_d)
        if need_ln1_bias:
            bqk = T(perm, [1, 2048], bf16, "bqk")
            nc.sync.dma_start(out=bqk, in_=bqk_d)
            bv = T(perm, [1, 1024], bf16, "bv")
            nc.sync.dma_start(out=bv, in_=bv_d)

        def layernorm_fm(get_src, dst_write, ntch, pps, paux):
            """Feature-major LN -> fp8 dst. get_src(c, tcols) -> fp32 [P,512]
            slice; dst_write(c, t) -> fp8 dst slice."""
            for t in range(ntch):
                tc512 = ts(t, 512)
                xs = [get_src(c, tc512) for c in range(8)]
                s_ps = T(pps, [1, 512], f32, "ps")
                ss_ps = T(pps, [1, 512], f32, "ps")
                for c in range(8):
                    xb = T(sq_pool, [P, 512], bf16, "xb")
                    nc.gpsimd.tensor_copy(out=xb, in_=xs[c])
                    sq = T(sq_pool, [P, 512], bf16, "sq")
                    nc.scalar.activation(out=sq, in_=xs[c], func=AF.Square)
                    nc.tensor.matmul(s_ps, ones128b, xb,
                                     start=(c == 0), stop=(c == 7),
                                     skip_group_check=True)
                    nc.tensor.matmul(ss_ps, ones128b, sq,
                                     start=(c == 0), stop=(c == 7),
                                     skip_group_check=True)
                mean = T(misc, [1, 512], bf16, "stat", bufs=6)
                nc.scalar.activation(out=mean, in_=s_ps, func=AF.Copy,
                                     scale=1.0 / DIM)
                msq = T(misc, [1, 512], f32, "stat", bufs=6)
                nc.scalar.activation(out=msq, in_=mean, func=AF.Square)
                var = T(misc, [1, 512], f32, "stat", bufs=6)
                nc.vector.scalar_tensor_tensor(
                    out=var, in0=ss_ps, scalar=1.0 / DIM, in1=msq,
                    op0=ALU.mult, op1=ALU.subtract)
                std = T(misc, [1, 512], f32, "stat", bufs=6)
                nc.scalar.activation(out=std, in_=var, func=AF.Sqrt,
                                     bias=eps_ap)
                rstd = T(misc, [1, 512], f32, "stat", bufs=6)
                nc.vector.reciprocal_approx_fast(out=rstd, in_=std)
                mb_ps = T(paux, [P, 512], f32, "aux")
                nc.tensor.matmul(mb_ps, ones1b, mean, start=True, stop=True)
                rb_ps = T(paux, [P, 512], f32, "aux")
                nc.tensor.matmul(rb_ps, ones1f, rstd, start=True, stop=True)
                mbs = T(misc, [P, 512], bf16, "mbs")
                nc.scalar.activation(out=mbs, in_=mb_ps, func=AF.Copy)
                rbs = T(misc, [P, 512], bf16, "rbs")
                nc.scalar.activation(out=rbs, in_=rb_ps, func=AF.Copy)
                for c in range(8):
                    tmp = T(sq_pool, [P, 512], bf16, "lntmp")
                    if c % 2 == 0:
                        nc.gpsimd.tensor_sub(tmp, xs[c], mbs)
                    else:
                        nc.vector.tensor_sub(tmp, xs[c], mbs)
                    nc.vector.tensor_mul(dst_write(c, t), tmp, rbs)

        # ================= P1: LN1 (x streamed from DRAM) ==============
        ps = tc.alloc_tile_pool(name="ps", bufs=5, space="PSUM")
        aux = tc.alloc_tile_pool(name="aux", bufs=3, space="PSUM")

        h1_pool = tc.alloc_tile_pool(name="h1", bufs=1, side="left")
        h1 = [T(h1_pool, [P, 2 * TOKH], fp8, f"h1_{i}") for i in range(4)]
        h1v = [h.rearrange("p (s t) -> p s t", s=2) for h in h1]

        xts_cache = {}

        def ln1_src(c, tcols):
            key = (c, tcols.start)
            if key not in xts_cache:
                xt = T(xts_pool, [P, 512], f32, "xts")
                nc.sync.dma_start(out=xt, in_=x_td[ts(c, P), tcols])
                xts_cache[key] = xt
            return xts_cache[key]

        def ln1_dst(c, t):
            return h1v[c // 2][:, c % 2, ts(t, 512)]

        layernorm_fm(ln1_src, ln1_dst, 3, ps, aux)

        # ================= P2: Q,K projections + RoPE ==================
        att_pool = tc.alloc_tile_pool(name="att", bufs=1, side="right")
        qhat = [T(att_pool, [P, TOK], bf16, f"qh_{f}") for f in range(8)]
        khat = [T(att_pool, [P, TOKH], bf16, f"kh_{f}") for f in range(8)]
        vvp = [T(att_pool, [P, 2 * 1056], fp8, f"vv_{k}") for k in range(6)]
        vvpv = [v.rearrange("p (s h x) -> p s h x", s=2, h=16) for v in vvp]

        def rope(src_ps, dst, dcols, cos, sin, ccols):
            qsb = T(rtmp, [P, 512], bf16, "qsb")
            nc.vector.tensor_copy(out=qsb, in_=src_ps)
            rot_ps = T(ps, [P, 512], f32, "ps")
            nc.tensor.matmul(rot_ps, rotm, qsb, start=True, stop=True)
            s1 = T(rtmp, [P, 512], bf16, "s1")
            nc.vector.tensor_mul(s1, qsb, cos[:, ccols])
            s2 = T(rtmp, [P, 512], bf16, "s2")
            nc.vector.tensor_mul(s2, rot_ps, sin[:, ccols])
            nc.vector.tensor_add(dst[:, dcols], s1, s2)

        for fp in range(8):  # 256-wide fout stripes; 0-3 = q, 4-7 = k
            wt = T(wpool, [P, 2048], fp8, "w")
            nc.sync.dma_start(out=wt, in_=wqk_d[:, fp])
            wtv = wt.rearrange("p (kp s m) -> p kp s m", kp=4, s=2)
            for sub in range(2):
                f = 2 * fp + sub
                is_q = fp < 4
                for t in range(2 if is_q else 3):
                    qps = T(ps, [P, 512], f32, "ps")
                    hcol = slice(512 + t * 512, 1024 + t * 512) if is_q \
                        else slice(t * 512, (t + 1) * 512)
                    for kp in range(4):
                        nc.tensor.matmul(
                            qps, wtv[:, kp, :, ts(sub, P)],
                            h1v[kp][:, :, hcol],
                            start=(kp == 0),
                            stop=(kp == 3) and not need_ln1_bias,
                            perf_mode=DR)
                    if need_ln1_bias:
                        nc.tensor.matmul(qps, bqk[:, ts(f, P)], onesr,
                                         start=False, stop=True)
                    if is_q:
                        rope(qps, qhat[f], ts(t, 512), cosq, sinq, ts(t, 512))
                    else:
                        rope(qps, khat[f - 8], ts(t, 512), cosk, sink,
                             ts(t, 512))

        # ================= P2b: V (token-major, fp8 x16) ===============
        for k in range(6):
            nc.gpsimd.memset(vvp[k], OS)  # col 64 of each 66-slot = 16 (sums)
        wvt = []
        for kp in range(4):
            wt = T(wpool, [P, 2048], fp8, "w")
            nc.sync.dma_start(out=wt, in_=wv_d[:, kp])
            wvt.append(wt.rearrange("p (s m) -> p s m", s=2))
        for k in range(12):  # token chunks; vv pairs fill in order
            for vh in range(2):
                vps = T(ps, [P, 512], f32, "ps")
                for kp in range(4):
                    nc.tensor.matmul(
                        vps, h1v[kp][:, :, ts(k, P)],
                        wvt[kp][:, :, ts(vh, 512)],
                        start=(kp == 0),
                        stop=(kp == 3) and not need_ln1_bias,
                        perf_mode=DR)
                if need_ln1_bias:
                    nc.tensor.matmul(vps, ones1b, bv[:, ts(vh, 512)],
                                     start=False, stop=True)
                dst = vvpv[k // 2][:, k % 2, 8 * vh:8 * vh + 8, 0:64]
                vpsv = vps.rearrange("p (h x) -> p h x", h=8)
                nc.vector.tensor_scalar_mul(dst, vpsv, 1.0 / 4.0)
        h1_pool.release()

        # ================= P3: windowed attention ======================
        oh_pool = tc.alloc_tile_pool(name="oh", bufs=1, side="left")
        op = [T(oh_pool, [P, 2 * TOK], fp8, f"op_{j}") for j in range(4)]
        opv = [o.rearrange("p (s t) -> p s t", s=2) for o in op]
        ptt = [T(oh_pool, [P, 1024], fp8, f"pt_{i}") for i in range(8)]
        for i in range(8):
            nc.gpsimd.memset(ptt[i], 0.0)
        pttv = [p.rearrange("p (s q) -> p s q", s=2) for p in ptt]
        wo_t = []
        for d in range(8):  # prefetch out-proj stripes during attention
            wt = T(wpool, [P, 1024], fp8, "w")
            nc.sync.dma_start(out=wt, in_=wo_d[:, d])
            wo_t.append(wt.rearrange("p (j s m) -> p j s m", j=4, s=2))

        aux.release()
        ps.release()
        spool = tc.alloc_tile_pool(name="spool", bufs=2, space="PSUM")
        aux3 = tc.alloc_tile_pool(name="aux3", bufs=3, space="PSUM")

        for lw in range(2):
            for hp in range(8):
                o_ps = [T(aux3, [66, 512], f32, "ops", bufs=3) for _ in range(2)]
                for pg in range(4):
                    halo = pg < 2
                    sp = [T(spool, [P, 1024], f32, "sp") for _ in range(2)]
                    for gi in range(2):
                        g = 2 * pg + gi
                        own = g >= 4
                        q0 = P * (g - 4) if own else 0
                        for hi in range(2):
                            hb = 64 * hi
                            nc.tensor.matmul(
                                sp[hi][:, gi * 512 + q0:(gi + 1) * 512],
                                khat[hp][hb:hb + 64, ts(lw * 4 + g, P)],
                                qhat[hp][hb:hb + 64,
                                         lw * 512 + q0:(lw + 1) * 512],
                                start=True, stop=not own,
                                skip_group_check=True)
                        if own:
                            for hi in range(2):
                                nc.tensor.matmul(
                                    sp[hi][:, gi * 512 + q0:
                                           gi * 512 + q0 + P],
                                    id128, tri, start=False, stop=True,
                                    skip_group_check=True)
                    for hi in range(2):
                        k8 = pg * 2 + hi
                        if halo:
                            bias = pb if lw == 0 else 0.0
                            nc.scalar.activation(
                                out=ptt[k8][:, 0:1024], in_=sp[hi][:, 0:1024],
                                func=AF.Exp, bias=bias)
                        else:
                            for gi in range(2):
                                q0 = P * (2 * pg + gi - 4)
                                nc.scalar.activation(
                                    out=ptt[k8][:, gi * 512 + q0:
                                                (gi + 1) * 512],
                                    in_=sp[hi][:, gi * 512 + q0:
                                               (gi + 1) * 512],
                                    func=AF.Exp)
                    gp = lw * 2 + pg
                    qmin = 0 if halo else P * (2 * pg - 4)
                    for hi in range(2):
                        h = 2 * hp + hi
                        k8 = pg * 2 + hi
                        nc.tensor.matmul(
                            o_ps[hi][:, qmin:512],
                            vvpv[gp][:, :, h, 0:66],
                            pttv[k8][:, :, qmin:512],
                            start=(pg == 0), stop=(pg == 3),
                            perf_mode=DR, skip_group_check=True)
                # ---- normalize both heads; write packed fp8 o (x16) ----
                ssb2 = T(misc, [33, 512], f32, "ssb2")
                nc.vector.memset(ssb2, 1.0)
                nc.vector.tensor_copy(ssb2[0:1, :], o_ps[0][64:65, :])
                nc.vector.tensor_copy(ssb2[32:33, :], o_ps[1][64:65, :])
                bcr2 = T(misc, [33, 512], f32, "bcr2")
                nc.vector.reciprocal_approx_fast(out=bcr2, in_=ssb2)
                bcb = T(aux3, [P, 512], f32, "bcb", bufs=1)
                nc.tensor.matmul(bcb, selpair, bcr2, start=True, stop=True)
                bcs = T(misc, [P, 512], bf16, "bcs", bufs=2)
                nc.scalar.activation(out=bcs, in_=bcb, func=AF.Copy)
                for hi in range(2):
                    h = 2 * hp + hi
                    j, ko = h // 4, (h % 4) // 2
                    dst = opv[j][64 * hi:64 * hi + 64, ko, ts(lw, 512)]
                    nc.vector.tensor_mul(dst, o_ps[hi][0:64, :],
                                         bcs[64 * hi:64 * hi + 64, :])
        att_pool.release()
        aux3.release()
        spool.release()
        ps2 = tc.alloc_tile_pool(name="ps2", bufs=5, space="PSUM")
        aux2 = tc.alloc_tile_pool(name="aux2", bufs=3, space="PSUM")

        # ================= P4: output proj + residual ==================
        x2_pool = tc.alloc_tile_pool(name="x2", bufs=1, side="right")
        x2 = [T(x2_pool, [P, TOK], f32, f"x2_{c}") for c in range(8)]
        for d in range(8):
            wt = wo_t[d]
            for t in range(2):
                yps = T(ps2, [P, 512], f32, "ps")
                for j in range(4):
                    nc.tensor.matmul(
                        yps, wt[:, j], opv[j][:, :, ts(t, 512)],
                        start=(j == 0), stop=(j == 3), perf_mode=DR)
                xo = T(xts_pool, [P, 512], f32, "xts")
                nc.sync.dma_start(
                    out=xo,
                    in_=x_td[ts(d, P), 512 + t * 512: 1024 + t * 512])
                nc.vector.scalar_tensor_tensor(
                    out=x2[d][:, ts(t, 512)], in0=yps,
                    scalar=1.0 / (WS * OS), in1=xo,
                    op0=ALU.mult, op1=ALU.add)
        oh_pool.release()

        # ================= P5: LN2 =====================================
        h2_pool = tc.alloc_tile_pool(name="h2", bufs=1, side="left")
        h2 = [T(h2_pool, [P, 2 * TOK], fp8, f"h2_{i}") for i in range(4)]
        h2v = [h.rearrange("p (s t) -> p s t", s=2) for h in h2]

        def ln2_src(c, tcols):
            return x2[c][:, tcols]

        def ln2_dst(c, t):
            return h2v[c // 2][:, c % 2, ts(t, 512)]

        layernorm_fm(ln2_src, ln2_dst, 2, ps2, aux2)

        # ================= P6: FFN (GEGLU, fp8 DoubleRow) ==============
        e_pool = tc.alloc_tile_pool(name="e", bufs=1, side="right")
        ep = [T(e_pool, [P, 2 * TOK], fp8, f"e_{i}") for i in range(NIC // 2)]
        epv = [e.rearrange("p (s t) -> p s t", s=2) for e in ep]
        for i in range(NIC):
            wa = T(wpool, [P, 1024], fp8, "w")
            nc.sync.dma_start(out=wa, in_=wf1a_d[:, i])
            wav = wa.rearrange("p (kp s m) -> p kp s m", kp=4, s=2)
            wg = T(wpool, [P, 1024], fp8, "w")
            nc.sync.dma_start(out=wg, in_=wf1g_d[:, i])
            wgv = wg.rearrange("p (kp s m) -> p kp s m", kp=4, s=2)
            for t in range(2):
                aps = T(ps2, [P, 512], f32, "ps")
                gps = T(ps2, [P, 512], f32, "ps")
                for kp in range(4):
                    nc.tensor.matmul(aps, wav[:, kp],
                                     h2v[kp][:, :, ts(t, 512)],
                                     start=(kp == 0), stop=(kp == 3),
                                     perf_mode=DR)
                for kp in range(4):
                    nc.tensor.matmul(gps, wgv[:, kp],
                                     h2v[kp][:, :, ts(t, 512)],
                                     start=(kp == 0), stop=(kp == 3),
                                     perf_mode=DR)
                gt = T(gtp, [P, 512], bf16, "gt")
                nc.scalar.activation(out=gt, in_=gps, func=AF.Gelu,
                                     scale=1.0 / WS)
                nc.vector.scalar_tensor_tensor(
                    out=epv[i // 2][:, i % 2, ts(t, 512)], in0=aps,
                    scalar=OS / WS, in1=gt, op0=ALU.mult, op1=ALU.mult)
        h2_pool.release()

        # ff2 + residual + store
        for d in range(8):
            wt = T(wpool, [P, 2816], fp8, "w")
            nc.sync.dma_start(out=wt, in_=wf2_d[:, d])
            wtv = wt.rearrange("p (kp s m) -> p kp s m", kp=11, s=2)
            for t in range(2):
                yps = T(ps2, [P, 512], f32, "ps")
                for kp in range(11):
                    nc.tensor.matmul(yps, wtv[:, kp],
                                     epv[kp][:, :, ts(t, 512)],
                                     start=(kp == 0), stop=(kp == 10),
                                     perf_mode=DR)
                ot = T(otp, [P, 512], f32, "ot")
                nc.vector.scalar_tensor_tensor(
                    out=ot, in0=yps, scalar=1.0 / (WS * OS),
                    in1=x2[d][:, ts(t, 512)], op0=ALU.mult, op1=ALU.add)
                nc.sync.dma_start(out=out_td[ts(d, P), ts(t, 512)], in_=ot)
        e_pool.release()
        x2_pool.release()

        aux2.release()
        ps2.release()
        otp.release()
        gtp.release()
        rtmp.release()
        sq_pool.release()
        xts_pool.release()
        misc.release()
        wpool.release()
        perm.release()

    nc.compile()
    return nc


def _host_prep(inputs):
    x = np.asarray(inputs["x"], np.float32)
    ln1_w = np.asarray(inputs["ln1_w"], np.float32)
    ln1_b = np.asarray(inputs["ln1_b"], np.float32)
    w_qkv = np.asarray(inputs["w_qkv"], np.float32)
    w_out = np.asarray(inputs["w_out"], np.float32)
    ln2_w = np.asarray(inputs["ln2_w"], np.float32)
    w_ff1 = np.asarray(inputs["w_ff1"], np.float32)
    w_ff2 = np.asarray(inputs["w_ff2"], np.float32)

    E4 = ml_dtypes.float8_e4m3

    need_ln1_bias = bool(np.any(ln1_b != 0.0))

    wq_eff = (w_qkv * ln1_w[:, None]) * WS
    wqk = wq_eff[:, :2048]
    wv = wq_eff[:, 2048:]

    def pack_pairs(w, nstripe, mwid):
        """w [1024, nstripe*mwid] -> [128, nstripe, 8*mwid] fp8 with layout
        [p, stripe, (kp, s, m)] = w[(2kp+s)*128+p, stripe*mwid+m]."""
        kin = w.shape[0]
        nkp = kin // 256
        a = w.reshape(nkp, 2, P, nstripe, mwid)  # [kp, s, p, stripe, m]
        a = a.transpose(2, 3, 0, 1, 4)  # [p, stripe, kp, s, m]
        return np.ascontiguousarray(
            a.reshape(P, nstripe, nkp * 2 * mwid)).astype(E4)

    # wqk: stripes of 256 out-features; [p, fp, (kp, s, m256)]
    a = wqk.reshape(4, 2, P, 8, 256)  # [kp, s, p, fp, m]
    wqk8 = np.ascontiguousarray(
        a.transpose(2, 3, 0, 1, 4).reshape(P, 8, 2048)).astype(E4)
    # wv: [p, kp, (s, m1024)]
    a = wv.reshape(4, 2, P, 1024)  # [kp, s, p, m]
    wv8 = np.ascontiguousarray(
        a.transpose(2, 0, 1, 3).reshape(P, 4, 2048)).astype(E4)
    # wo: [ki, d, (j, ko, m)] = wo[j*256+ko*128+ki, d*128+m]
    a = (w_out * WS).reshape(4, 2, P, 8, P)  # [j, ko, ki, d, m]
    wo8 = np.ascontiguousarray(
        a.transpose(2, 3, 0, 1, 4).reshape(P, 8, 1024)).astype(E4)

    wf1_eff = w_ff1 * ln2_w[:, None] * WS
    wf1a = np.zeros((DIM, INNER_PAD), np.float32)
    wf1a[:, :INNER] = wf1_eff[:, :INNER]
    wf1g = np.zeros((DIM, INNER_PAD), np.float32)
    wf1g[:, :INNER] = wf1_eff[:, INNER:]
    wf1a8 = pack_pairs(wf1a, NIC, P)
    wf1g8 = pack_pairs(wf1g, NIC, P)

    wf2 = np.zeros((INNER_PAD, DIM), np.float32)
    wf2[:INNER] = w_ff2 * WS
    a = wf2.reshape(11, 2, P, 8, P)  # [kp, s, p, d, m]
    wf28 = np.ascontiguousarray(
        a.transpose(2, 3, 0, 1, 4).reshape(P, 8, 2816)).astype(E4)

    rot64 = np.zeros((DH, DH), np.float32)
    for m in range(32):
        rot64[m + 32, m] = -1.0
        rot64[m, m + 32] = 1.0
    rotm = np.zeros((P, P), np.float32)
    rotm[:DH, :DH] = rot64
    rotm[DH:, DH:] = rot64
    rotm = rotm.astype(BF)

    tri = np.where(np.arange(P)[None, :] >= np.arange(P)[:, None],
                   0.0, NEG).astype(BF)  # [key-in-diag, q]
    idm = np.eye(P, dtype=np.float32).astype(BF)

    b_qkv = (ln1_b @ w_qkv).astype(np.float32) * WS  # [3072]
    b_qk = np.ascontiguousarray(b_qkv[None, :2048]).astype(BF)
    b_v = np.ascontiguousarray(b_qkv[None, 2048:]).astype(BF)

    inv = 1.0 / (10000.0 ** (np.arange(0, DH, 2, dtype=np.float64) / DH))

    in_maps = []
    for c in range(NCORES):
        b = c // 4
        pos0 = (c % 4) * TOK
        xs = np.zeros((TOKH, DIM), np.float32)
        lo = pos0 - W
        if lo < 0:
            xs[W:] = x[b, pos0:pos0 + TOK]
        else:
            xs[:] = x[b, lo:pos0 + TOK]
        x_t = np.ascontiguousarray(xs.T)

        pos_own = pos0 + np.arange(TOK, dtype=np.float64)
        pos_kv = pos0 - W + np.arange(TOKH, dtype=np.float64)

        def cs(pos, scale):
            f = pos[None, :] * inv[:, None]  # [32, T]
            f = np.concatenate([f, f], axis=0)  # [64, T]
            co = np.tile(np.cos(f) * scale, (2, 1)).astype(BF)
            si = np.tile(np.sin(f) * scale, (2, 1)).astype(BF)
            return np.ascontiguousarray(co), np.ascontiguousarray(si)

        cos_q, sin_q = cs(pos_own, DH ** -0.5 / WS)
        cos_k, sin_k = cs(pos_kv, 1.0 / WS)

        pbias = np.full((P, 1), NEG if c % 4 == 0 else 0.0, np.float32)

        m = {
            "x_t": x_t, "wqk8": wqk8, "wv8": wv8, "wo8": wo8,
            "wf1a8": wf1a8, "wf1g8": wf1g8, "wf28": wf28,
            "cos_q": cos_q, "sin_q": sin_q, "cos_k": cos_k, "sin_k": sin_k,
            "tri": tri, "idm": idm, "pbias": pbias, "rotm": rotm,
        }
        if need_ln1_bias:
            m["b_qk"] = b_qk
            m["b_v"] = b_v
        in_maps.append(m)
    return in_maps, need_ln1_bias


def get_program_and_inputs(inputs):
    in_maps, need_ln1_bias = _host_prep(inputs)
    key = ("prog", need_ln1_bias)
    if key not in _CACHE:
        _CACHE[key] = _build_program(need_ln1_bias)
    return _CACHE[key], in_maps


def assemble(results):
    out = np.empty((B, N, DIM), np.float32)
    for c in range(NCORES):
        b = c // 4
        pos0 = (c % 4) * TOK
        out[b, pos0:pos0 + TOK] = results[c]["out_t"].T
    return out


def run(inputs, trace=False, **kw):
    from concourse.bass_utils import run_bass_kernel_spmd
    nc, in_maps = get_program_and_inputs(inputs)
    res = run_bass_kernel_spmd(nc, in_maps, list(range(NCORES)),
                               trace=trace, **kw)
    return assemble(res.results), res


def kernel(**inputs) -> np.ndarray:
    out, _ = run(inputs, trace=False)
    return out
